# revision 9
# baseline (speedup 1.0000x reference)
"""Causal multi-head attention (B=2, T=2048, DIM=2048, H=16, HD=128) on 8
Trainium2 NeuronCores.

Sharding: core = 4*b + g  (b = batch 0..1, g = head-group 0..3, 4 heads each).
Each core computes, for its batch b and heads 4g..4g+3:
  QKV projection -> causal attention -> partial out = attn_out @ wo[rows of g]
The host sums the 4 partial outputs per batch (the "all-reduce after wo").

On-device layout avoids every transpose:
  - host passes x[b].T, so projections contract d with d on partitions
  - Q^T/K^T kept as [hd, t] (head dim on partitions)
  - scores computed as S^T = K^T_tile.T @ Q^T  ([j, i] layout)
  - exp via ScalarE; causal masking = multiply diagonal tiles by 0/1 masks
  - P@V computed as O^T via lhsT = V tile (natural [t, hd] layout)
  - denominator via ones-vector matmul, normalization via GPSIMD
    partition-broadcast of 1/d + VectorE multiply
  - wo projection consumes O^T tiles directly as stationary operands
All matmul operands are bf16: same PE streaming rate as float32r, but the
weight loads legalize into separate LDWEIGHTS instructions that the PE's
reorder window overlaps with the previous matmul (float32r must self-load
serially, costing ~77ns per matmul). QKV partial sums across the four
d-tile streaming groups accumulate in fp32 SBUF so the bf16 rounding is
paid once, on the final value.
"""

import math
import os

import numpy as np

B, T, D, H, HD = 2, 2048, 2048, 16, 128
NH = 4            # heads per core
NCORES = 8
TCH = 512         # query-chunk width (moving-operand free size)
NDT = D // 128    # 16 d-tiles (contraction tiles for projections)
NTT = T // 128    # 16 t-tiles
NCH = T // TCH    # 4 query chunks
DQ = 4            # d-tiles per accumulation group (PSUM chain length)
NQ = NDT // DQ    # 4 groups

_BUILT = {}
LAST_RESULTS = None  # BassKernelResults of the most recent kernel() call


def _build(causal: bool):
    import concourse.mybir as mybir
    import concourse.tile as tile
    from concourse import bacc

    F32 = mybir.dt.float32
    BF16 = mybir.dt.bfloat16
    EXP = mybir.ActivationFunctionType.Exp
    scale = 1.0 / math.sqrt(HD)

    nc = bacc.Bacc(None, name="attn")
    xT = nc.dram_tensor("xT", [D, T], BF16, kind="ExternalInput")
    wqkv = nc.dram_tensor("wqkv", [D, 3 * NH * HD], BF16, kind="ExternalInput")
    wo = nc.dram_tensor("wo", [NH * HD, D], BF16, kind="ExternalInput")
    masks = nc.dram_tensor("masks", [128, 4 * TCH], BF16, kind="ExternalInput")
    if not causal:
        maskT = nc.dram_tensor("maskT", [T, T], BF16, kind="ExternalInput")
    out = nc.dram_tensor("out", [T, D], F32, kind="ExternalOutput")

    with tile.TileContext(nc) as tc:
        with (
            tc.tile_pool(name="persist", bufs=1) as persist,
            tc.tile_pool(name="ps3", bufs=3, space="PSUM") as ps3,
            tc.tile_pool(name="pso4", bufs=4, space="PSUM") as pso4,
            tc.tile_pool(name="ps1", bufs=1, space="PSUM") as ps1,
        ):
            # persistent operands for the attention phase
            qkt = persist.tile([128, 8, T], BF16)          # slots 0-3: Q^T heads, 4-7: K^T heads
            vsb = persist.tile([128, NTT, NH * HD], BF16)  # V, [t-tile][local t, head*hd]
            ones_f = persist.tile([128, 1], F32)
            ones = persist.tile([128, 1], BF16)
            nc.vector.memset(ones_f[:], 1.0)
            nc.vector.tensor_copy(ones[:], ones_f[:])
            # dummy broadcast: preload the GpSimd PartitionBroadcast ucode
            # library now (~11us HBM fetch) so the first real normalize
            # doesn't stall the whole attention pipeline on LIBRARY_RELOAD
            warm = persist.tile([128, 1], F32)
            nc.gpsimd.partition_broadcast(warm[:], ones_f[0:1, :])

            # ---- Phase A: QKV projections, streaming x^T / wqkv d-tiles.
            # First groups are small so compute starts early. Partial sums
            # across groups accumulate in fp32 SBUF (exact); the last
            # group's add writes the bf16 copy used by the matmuls. ----
            groups = [2, 4, 5, 5]
            offs = [sum(groups[:i]) for i in range(len(groups))]
            comb_n = [0]
            pa_n = [0]

            def pa_ps(shape):
                # phase A: alternate chain outputs across both PSUM pools so
                # seven banks rotate instead of three — deeper drain overlap
                pa_n[0] += 1
                pool = ps3 if pa_n[0] % 2 else pso4
                tag = "ps_s" if pa_n[0] % 2 else "ps_o"
                return pool.tile(shape, F32, tag=tag, name=f"pa{pa_n[0] % 2}")

            with (
                tc.tile_pool(name="acc", bufs=1) as accp,
                tc.tile_pool(name="xw", bufs=8) as xw,
            ):
                qkt32 = accp.tile([128, 8, T], F32)
                vsb32 = accp.tile([128, NTT, NH * HD], F32)
                for qg, (off, dq) in enumerate(zip(offs, groups)):
                    first, last = qg == 0, qg == len(groups) - 1
                    xts, wqks, wvs = [], [], []
                    for k in range(dq):
                        di = off + k
                        xt_t = xw.tile([128, T], BF16, tag="xt")
                        nc.sync.dma_start(xt_t[:], xT[di * 128:(di + 1) * 128, :])
                        wv_t = xw.tile([128, NH * HD], BF16, tag="wv", bufs=7)
                        nc.sync.dma_start(wv_t[:],
                                          wqkv[di * 128:(di + 1) * 128,
                                               2 * NH * HD:3 * NH * HD])
                        xts.append(xt_t)
                        wvs.append(wv_t)
                    # wqk after the group's xt/wv: the V chains only need
                    # xt+wv, so the first compute starts ~2.5us earlier
                    for k in range(dq):
                        di = off + k
                        wqk_t = xw.tile([128, 2 * NH * HD], BF16, tag="wqk")
                        nc.sync.dma_start(wqk_t[:],
                                          wqkv[di * 128:(di + 1) * 128, 0:2 * NH * HD])
                        wqks.append(wqk_t)
                    # V first: attention's PV chains need V earliest
                    for tt in range(NTT):
                        ps = pa_ps([128, NH * HD])
                        for k in range(dq):
                            nc.tensor.matmul(
                                ps[:],
                                xts[k][:, tt * 128:(tt + 1) * 128],
                                wvs[k][:],
                                start=(k == 0),
                                stop=(k == dq - 1),
                            )
                        acc = vsb32[:, tt, :]
                        if first:
                            comb_n[0] += 1
                            if comb_n[0] % 2 == 0:
                                nc.scalar.copy(acc, ps[:])
                            else:
                                nc.vector.tensor_copy(acc, ps[:])
                        elif last:
                            nc.vector.tensor_add(vsb[:, tt, :], acc, ps[:])
                        else:
                            nc.vector.tensor_add(acc, acc, ps[:])
                    # Q^T / K^T, ordered so K0/Q0 chunk 0 completes first
                    for tch in range(NCH):
                        for s in (4, 0, 5, 1, 6, 2, 7, 3):
                            ps = pa_ps([128, TCH])
                            for k in range(dq):
                                nc.tensor.matmul(
                                    ps[:],
                                    wqks[k][:, s * 128:(s + 1) * 128],
                                    xts[k][:, tch * TCH:(tch + 1) * TCH],
                                    start=(k == 0),
                                    stop=(k == dq - 1),
                                )
                            acc = qkt32[:, s, tch * TCH:(tch + 1) * TCH]
                            if first:
                                comb_n[0] += 1
                                if comb_n[0] % 2 == 0:
                                    nc.scalar.copy(acc, ps[:])
                                else:
                                    nc.vector.tensor_copy(acc, ps[:])
                            elif last:
                                nc.vector.tensor_add(
                                    qkt[:, s, tch * TCH:(tch + 1) * TCH], acc, ps[:])
                            else:
                                nc.vector.tensor_add(acc, acc, ps[:])

            # Phase B/C pools open after the stream pool closed, reusing its
            # SBUF region. wo loads land in that freed space.
            with (
                tc.tile_pool(name="post", bufs=1) as post,
                tc.tile_pool(name="work", bufs=5) as work,
                tc.tile_pool(name="sml", bufs=2) as sml,
                tc.tile_pool(name="otp", bufs=2) as otp,
                tc.tile_pool(name="outp", bufs=4) as outp,
            ):
                msb = post.tile([128, 4 * TCH], BF16)  # diagonal causal masks
                nc.sync.dma_start(msb[:], masks[:])
                # separate 2D tiles: a DMA into a 3D middle-index slice
                # ([128, 1, D] dst AP) hard-faults the exec unit
                wosb = []
                for et in range(NH):
                    wt_ = post.tile([128, D], BF16, tag=f"wos{et}")
                    nc.sync.dma_start(wt_[:], wo[et * 128:(et + 1) * 128, :])
                    wosb.append(wt_)

                # ---- Phase B+C: attention per chunk, with the previous
                # chunk's wo-projection chains interleaved into the jt loop so
                # the in-order PE never sits on PSUM slot recycling ----
                pc_n = [0]

                def emit_pc_chain(c0, lt, oc, otc0, alternate=False):
                    if alternate and pc_n[0] % 2 == 0:
                        ps = ps3.tile([128, TCH], F32, tag="ps_s")
                    else:
                        ps = pso4.tile([128, TCH], F32, tag="ps_o")
                    for h2 in range(NH):
                        nc.tensor.matmul(
                            ps[:],
                            otc0[:, h2, lt * 128:(lt + 1) * 128],
                            wosb[h2][:, oc * TCH:(oc + 1) * TCH],
                            start=(h2 == 0),
                            stop=(h2 == NH - 1),
                        )
                    ost = outp.tile([128, TCH], F32, tag="ost")
                    pc_n[0] += 1
                    if alternate and pc_n[0] % 2 == 0:
                        nc.scalar.copy(ost[:], ps[:])
                    else:
                        nc.vector.tensor_copy(ost[:], ps[:])
                    nc.sync.dma_start(
                        out[(4 * c0 + lt) * 128:(4 * c0 + lt + 1) * 128,
                            oc * TCH:(oc + 1) * TCH],
                        ost[:],
                    )

                pending = []
                for c in range(NCH):
                    otc = otp.tile([128, NH, TCH], BF16, tag="ot")
                    njt = 4 * (c + 1) if causal else NTT
                    steps_total = NH * njt
                    spacing = max(1, steps_total // len(pending)) if pending else 0
                    step = 0
                    for h in range(NH):
                        pso = pso4.tile([128, TCH], F32, tag="ps_o")
                        psd = ps1.tile([1, TCH], F32, tag="ps_d")

                        def emit_pss(jt):
                            # diagonal tiles: queries below 128*qd are fully
                            # masked for this key tile — skip them in the
                            # score matmul. The exp still runs full-width
                            # (stale psum is finite; the mask mul zeroes it).
                            qd = jt - 4 * c if causal else -1
                            off = 128 * qd if qd > 0 else 0
                            pss = ps3.tile([128, TCH], F32, tag="ps_s")
                            nc.tensor.matmul(
                                pss[:, off:],
                                qkt[:, 4 + h, jt * 128:(jt + 1) * 128],
                                qkt[:, h, c * TCH + off:(c + 1) * TCH],
                                start=True,
                                stop=True,
                            )
                            pt = work.tile([128, TCH], BF16, tag="pt")
                            nc.scalar.activation(pt[:], pss[:], EXP, scale=scale)
                            if causal:
                                if qd >= 0:
                                    nc.vector.tensor_mul(pt[:], pt[:], msb[:, qd * TCH:(qd + 1) * TCH])
                            else:
                                mt = work.tile([128, TCH], BF16, tag="mt")
                                nc.sync.dma_start(
                                    mt[:],
                                    maskT[jt * 128:(jt + 1) * 128,
                                          c * TCH:(c + 1) * TCH],
                                )
                                nc.vector.tensor_mul(pt[:], pt[:], mt[:])
                            return pt

                        pts = {}
                        for jt in range(min(2, njt)):
                            pts[jt] = emit_pss(jt)
                        prev_pt = None
                        prev_ptp = None
                        for jt in range(njt):
                            if jt + 2 < njt:
                                pts[jt + 2] = emit_pss(jt + 2)
                            pt = pts.pop(jt)
                            qd2 = jt - 4 * c if causal else -1
                            voff = 128 * qd2 if qd2 > 0 else 0
                            nc.tensor.matmul(
                                pso[:, voff:],
                                vsb[:, jt, h * HD:(h + 1) * HD],
                                pt[:, voff:],
                                start=(jt == 0),
                                stop=(jt == njt - 1),
                            )
                            # denominator: sum pt quads on DVE (2x SBUF mode)
                            # and run the ones-matmul once per four tiles —
                            # quarters the PE cost of the d accumulation and,
                            # more importantly, halves the pipeline bubbles the
                            # 1-row ones-matmul punches into the MM stream
                            # (njt is always a multiple of 4)
                            if jt % 2 == 0:
                                prev_pt = pt
                            else:
                                ptp = work.tile([128, TCH], BF16, tag="ptp",
                                                bufs=3)
                                nc.vector.tensor_add(ptp[:], prev_pt[:], pt[:])
                                if jt % 4 == 1:
                                    prev_ptp = ptp
                                else:
                                    ptq = work.tile([128, TCH], BF16, tag="ptq")
                                    nc.vector.tensor_add(ptq[:], prev_ptp[:], ptp[:])
                                    nc.tensor.matmul(
                                        psd[:],
                                        ones[:, 0:1],
                                        ptq[:],
                                        start=(jt == 3),
                                        stop=(jt == njt - 1),
                                    )
                            step += 1
                            if pending and spacing and step % spacing == 0:
                                emit_pc_chain(*pending.pop(0))
                        # 1/d via single-op approx reciprocal (~18 bits, way
                        # beyond the bf16 pipeline); exact reciprocal costs
                        # 3.3us and Ln/Exp thrash the ACT table
                        drc = sml.tile([1, TCH], F32, tag="drc")
                        nc.vector.reciprocal_approx_fast(drc[:], psd[:])
                        bc = sml.tile([128, TCH], F32, tag="bc")
                        nc.gpsimd.partition_broadcast(bc[:], drc[:])
                        nc.vector.tensor_mul(otc[:, h, :], pso[:], bc[:])
                    while pending:
                        emit_pc_chain(*pending.pop(0))
                    pending = [(c, lt, oc, otc)
                               for lt in range(4) for oc in range(NCH)]
                # tail drain: alternate the PSUM->SBUF copies between DVE and
                # ScalarE so slot recycling isn't single-engine-latency-bound
                for chain in pending:
                    emit_pc_chain(*chain, alternate=True)
    nc.compile()
    return nc


def _get_built(causal: bool):
    if causal not in _BUILT:
        _BUILT[causal] = _build(causal)
    return _BUILT[causal]


def _diag_masks():
    # masks[jl, q*TCH + ii] = 1 if key (128*q + jl) <= query ii in the chunk
    q = np.arange(4)[:, None, None]
    jl = np.arange(128)[None, :, None]
    ii = np.arange(TCH)[None, None, :]
    m = (ii >= 128 * q + jl).astype(np.float32)        # [4, 128, TCH]
    return np.ascontiguousarray(m.transpose(1, 0, 2).reshape(128, 4 * TCH))


def kernel(x, mask, wqkv, wo):
    global LAST_RESULTS
    import ml_dtypes
    from concourse.bass_utils import run_bass_kernel_spmd

    bf16 = ml_dtypes.bfloat16
    x = np.ascontiguousarray(np.asarray(x, dtype=np.float32))
    wqkv_b = np.asarray(wqkv, dtype=np.float32).astype(bf16)
    wo_b = np.asarray(wo, dtype=np.float32).astype(bf16)
    mask_np = np.asarray(mask).reshape(T, T).astype(bool)
    causal = bool(np.array_equal(mask_np, np.tril(np.ones((T, T), dtype=bool))))

    nc = _get_built(causal)
    masks_arr = _diag_masks().astype(bf16)
    maskT = None
    if not causal:
        maskT = np.ascontiguousarray(mask_np.T.astype(np.float32)).astype(bf16)

    in_maps = []
    for core in range(NCORES):
        b, g = divmod(core, NH)
        xT = np.ascontiguousarray(x[b].T).astype(bf16)
        wq = wqkv_b[:, 0 * H * HD + g * NH * HD:0 * H * HD + (g + 1) * NH * HD]
        wk = wqkv_b[:, 1 * H * HD + g * NH * HD:1 * H * HD + (g + 1) * NH * HD]
        wv = wqkv_b[:, 2 * H * HD + g * NH * HD:2 * H * HD + (g + 1) * NH * HD]
        wqkv_g = np.ascontiguousarray(np.concatenate([wq, wk, wv], axis=1))
        wo_g = np.ascontiguousarray(wo_b[g * NH * HD:(g + 1) * NH * HD, :])
        m = {"xT": xT, "wqkv": wqkv_g, "wo": wo_g, "masks": masks_arr}
        if maskT is not None:
            m["maskT"] = maskT
        in_maps.append(m)

    trace = os.environ.get("ATTN_TRACE", "") not in ("", "0")
    res = run_bass_kernel_spmd(nc, in_maps, core_ids=list(range(NCORES)),
                               trace=trace)
    LAST_RESULTS = res

    acc = np.zeros((B, T, D), dtype=np.float64)
    for core in range(NCORES):
        b = core // NH
        acc[b] += res.results[core]["out"].astype(np.float64)
    return acc.astype(np.float32)


# revision 12
# speedup vs baseline: 1.1568x; 1.1568x over previous
"""Causal multi-head attention (B=2, T=2048, DIM=2048, H=16, HD=128) on 8
Trainium2 NeuronCores.

Sharding: core = 4*b + g  (b = batch 0..1, g = head-group 0..3, 4 heads each).
Each core computes, for its batch b and heads 4g..4g+3:
  QKV projection -> causal attention -> partial out = attn_out @ wo[rows of g]
The host sums the 4 partial outputs per batch (the "all-reduce after wo").

On-device layout avoids every transpose:
  - host passes x[b].T, so projections contract d with d on partitions
  - Q^T/K^T kept as [hd, t] (head dim on partitions)
  - scores computed as S^T = K^T_tile.T @ Q^T  ([j, i] layout)
  - exp via ScalarE; causal masking = multiply diagonal tiles by 0/1 masks
  - P@V computed as O^T via lhsT = V tile (natural [t, hd] layout)
  - denominator via ones-vector matmul, normalization via GPSIMD
    partition-broadcast of 1/d + VectorE multiply
  - wo projection consumes O^T tiles directly as stationary operands
All matmul operands are bf16: same PE streaming rate as float32r, but the
weight loads legalize into separate LDWEIGHTS instructions that the PE's
reorder window overlaps with the previous matmul (float32r must self-load
serially, costing ~77ns per matmul). QKV partial sums across the four
d-tile streaming groups accumulate in fp32 SBUF so the bf16 rounding is
paid once, on the final value.
"""

import math
import os

import numpy as np

B, T, D, H, HD = 2, 2048, 2048, 16, 128
NH = 4            # heads per core
NCORES = 8
TCH = 512         # query-chunk width (moving-operand free size)
NDT = D // 128    # 16 d-tiles (contraction tiles for projections)
NTT = T // 128    # 16 t-tiles
NCH = T // TCH    # 4 query chunks
DQ = 4            # d-tiles per accumulation group (PSUM chain length)
NQ = NDT // DQ    # 4 groups

_BUILT = {}
LAST_RESULTS = None  # BassKernelResults of the most recent kernel() call


def _build(causal: bool):
    import concourse.mybir as mybir
    import concourse.tile as tile
    from concourse import bacc

    F32 = mybir.dt.float32
    BF16 = mybir.dt.bfloat16
    EXP = mybir.ActivationFunctionType.Exp
    scale = 1.0 / math.sqrt(HD)

    nc = bacc.Bacc(None, name="attn")
    xT = nc.dram_tensor("xT", [D, T], BF16, kind="ExternalInput")
    wqkv = nc.dram_tensor("wqkv", [D, 3 * NH * HD], BF16, kind="ExternalInput")
    wo = nc.dram_tensor("wo", [NH * HD, D], BF16, kind="ExternalInput")
    masks = nc.dram_tensor("masks", [128, 4 * TCH], BF16, kind="ExternalInput")
    if not causal:
        maskT = nc.dram_tensor("maskT", [T, T], BF16, kind="ExternalInput")
    out = nc.dram_tensor("out", [T, D], F32, kind="ExternalOutput")

    with tile.TileContext(nc) as tc:
        with (
            tc.tile_pool(name="persist", bufs=1) as persist,
            tc.tile_pool(name="ps3", bufs=3, space="PSUM") as ps3,
            tc.tile_pool(name="pso4", bufs=4, space="PSUM") as pso4,
            tc.tile_pool(name="ps1", bufs=1, space="PSUM") as ps1,
        ):
            # persistent operands for the attention phase
            qkt = persist.tile([128, 8, T], BF16)          # slots 0-3: Q^T heads, 4-7: K^T heads
            vsb = persist.tile([128, NTT, NH * HD], BF16)  # V, [t-tile][local t, head*hd]
            ones_f = persist.tile([128, 1], F32)
            ones = persist.tile([128, 1], BF16)
            nc.vector.memset(ones_f[:], 1.0)
            nc.vector.tensor_copy(ones[:], ones_f[:])
            # dummy broadcast: preload the GpSimd PartitionBroadcast ucode
            # library now (~11us HBM fetch) so the first real normalize
            # doesn't stall the whole attention pipeline on LIBRARY_RELOAD
            warm = persist.tile([128, 1], F32)
            nc.gpsimd.partition_broadcast(warm[:], ones_f[0:1, :])

            # ---- Phase A: QKV projections, streaming x^T / wqkv d-tiles.
            # First groups are small so compute starts early. Partial sums
            # across groups accumulate in fp32 SBUF (exact); the last
            # group's add writes the bf16 copy used by the matmuls. ----
            groups = [2, 4, 5, 5]
            offs = [sum(groups[:i]) for i in range(len(groups))]
            comb_n = [0]
            with (
                tc.tile_pool(name="acc", bufs=1) as accp,
                tc.tile_pool(name="xw", bufs=8) as xw,
            ):
                qkt32 = accp.tile([128, 8, T], F32)
                vsb32 = accp.tile([128, NTT, NH * HD], F32)
                for qg, (off, dq) in enumerate(zip(offs, groups)):
                    first, last = qg == 0, qg == len(groups) - 1
                    xts, wqks, wvs = [], [], []
                    for k in range(dq):
                        di = off + k
                        xt_t = xw.tile([128, T], BF16, tag="xt")
                        nc.sync.dma_start(xt_t[:], xT[di * 128:(di + 1) * 128, :])
                        wv_t = xw.tile([128, NH * HD], BF16, tag="wv", bufs=7)
                        nc.sync.dma_start(wv_t[:],
                                          wqkv[di * 128:(di + 1) * 128,
                                               2 * NH * HD:3 * NH * HD])
                        xts.append(xt_t)
                        wvs.append(wv_t)
                    # wqk after the group's xt/wv: the V chains only need
                    # xt+wv, so the first compute starts ~2.5us earlier
                    for k in range(dq):
                        di = off + k
                        wqk_t = xw.tile([128, 2 * NH * HD], BF16, tag="wqk")
                        nc.sync.dma_start(wqk_t[:],
                                          wqkv[di * 128:(di + 1) * 128, 0:2 * NH * HD])
                        wqks.append(wqk_t)
                    # V first: attention's PV chains need V earliest
                    for tt in range(NTT):
                        ps = ps3.tile([128, NH * HD], F32, tag="ps_s")
                        for k in range(dq):
                            nc.tensor.matmul(
                                ps[:],
                                xts[k][:, tt * 128:(tt + 1) * 128],
                                wvs[k][:],
                                start=(k == 0),
                                stop=(k == dq - 1),
                            )
                        acc = vsb32[:, tt, :]
                        if first:
                            comb_n[0] += 1
                            if comb_n[0] % 2 == 0:
                                nc.scalar.copy(acc, ps[:])
                            else:
                                nc.vector.tensor_copy(acc, ps[:])
                        elif last:
                            nc.vector.tensor_add(vsb[:, tt, :], acc, ps[:])
                        else:
                            nc.vector.tensor_add(acc, acc, ps[:])
                    # Q^T / K^T, ordered so K0/Q0 chunk 0 completes first
                    for tch in range(NCH):
                        for s in (4, 0, 5, 1, 6, 2, 7, 3):
                            ps = ps3.tile([128, TCH], F32, tag="ps_s")
                            for k in range(dq):
                                nc.tensor.matmul(
                                    ps[:],
                                    wqks[k][:, s * 128:(s + 1) * 128],
                                    xts[k][:, tch * TCH:(tch + 1) * TCH],
                                    start=(k == 0),
                                    stop=(k == dq - 1),
                                )
                            acc = qkt32[:, s, tch * TCH:(tch + 1) * TCH]
                            if first:
                                comb_n[0] += 1
                                if comb_n[0] % 2 == 0:
                                    nc.scalar.copy(acc, ps[:])
                                else:
                                    nc.vector.tensor_copy(acc, ps[:])
                            elif last:
                                nc.vector.tensor_add(
                                    qkt[:, s, tch * TCH:(tch + 1) * TCH], acc, ps[:])
                            else:
                                nc.vector.tensor_add(acc, acc, ps[:])

            # Phase B/C pools open after the stream pool closed, reusing its
            # SBUF region. wo loads land in that freed space.
            with (
                tc.tile_pool(name="post", bufs=1) as post,
                tc.tile_pool(name="work", bufs=5) as work,
                tc.tile_pool(name="sml", bufs=2) as sml,
                tc.tile_pool(name="otp", bufs=2) as otp,
                tc.tile_pool(name="outp", bufs=4) as outp,
            ):
                msb = post.tile([128, 4 * TCH], BF16)  # diagonal causal masks
                nc.sync.dma_start(msb[:], masks[:])
                # separate 2D tiles: a DMA into a 3D middle-index slice
                # ([128, 1, D] dst AP) hard-faults the exec unit
                wosb = []
                for et in range(NH):
                    wt_ = post.tile([128, D], BF16, tag=f"wos{et}")
                    nc.sync.dma_start(wt_[:], wo[et * 128:(et + 1) * 128, :])
                    wosb.append(wt_)

                # ---- Phase B+C: attention per chunk, with the previous
                # chunk's wo-projection chains interleaved into the jt loop so
                # the in-order PE never sits on PSUM slot recycling ----
                pc_n = [0]

                def emit_pc_chain(c0, lt, oc, otc0, alternate=False):
                    if alternate and pc_n[0] % 2 == 0:
                        ps = ps3.tile([128, TCH], F32, tag="ps_s")
                    else:
                        ps = pso4.tile([128, TCH], F32, tag="ps_o")
                    for h2 in range(NH):
                        nc.tensor.matmul(
                            ps[:],
                            otc0[:, h2, lt * 128:(lt + 1) * 128],
                            wosb[h2][:, oc * TCH:(oc + 1) * TCH],
                            start=(h2 == 0),
                            stop=(h2 == NH - 1),
                        )
                    ost = outp.tile([128, TCH], F32, tag="ost")
                    pc_n[0] += 1
                    if alternate and pc_n[0] % 2 == 0:
                        nc.scalar.copy(ost[:], ps[:])
                    else:
                        nc.vector.tensor_copy(ost[:], ps[:])
                    nc.sync.dma_start(
                        out[(4 * c0 + lt) * 128:(4 * c0 + lt + 1) * 128,
                            oc * TCH:(oc + 1) * TCH],
                        ost[:],
                    )

                pending = []
                for c in range(NCH):
                    otc = otp.tile([128, NH, TCH], BF16, tag="ot")
                    njt = 4 * (c + 1) if causal else NTT
                    steps_total = NH * njt
                    spacing = max(1, steps_total // len(pending)) if pending else 0
                    step = 0
                    for h in range(NH):
                        pso = pso4.tile([128, TCH], F32, tag="ps_o")
                        psd = ps1.tile([1, TCH], F32, tag="ps_d")

                        def emit_pss(jt):
                            # diagonal tiles: queries below 128*qd are fully
                            # masked for this key tile — skip them in the
                            # score matmul. The exp still runs full-width
                            # (stale psum is finite; the mask mul zeroes it).
                            qd = jt - 4 * c if causal else -1
                            off = 128 * qd if qd > 0 else 0
                            pss = ps3.tile([128, TCH], F32, tag="ps_s")
                            nc.tensor.matmul(
                                pss[:, off:],
                                qkt[:, 4 + h, jt * 128:(jt + 1) * 128],
                                qkt[:, h, c * TCH + off:(c + 1) * TCH],
                                start=True,
                                stop=True,
                            )
                            pt = work.tile([128, TCH], BF16, tag="pt")
                            nc.scalar.activation(pt[:], pss[:], EXP, scale=scale)
                            if causal:
                                if qd >= 0:
                                    nc.vector.tensor_mul(pt[:], pt[:], msb[:, qd * TCH:(qd + 1) * TCH])
                            else:
                                mt = work.tile([128, TCH], BF16, tag="mt")
                                nc.sync.dma_start(
                                    mt[:],
                                    maskT[jt * 128:(jt + 1) * 128,
                                          c * TCH:(c + 1) * TCH],
                                )
                                nc.vector.tensor_mul(pt[:], pt[:], mt[:])
                            return pt

                        pts = {}
                        for jt in range(min(2, njt)):
                            pts[jt] = emit_pss(jt)
                        prev_pt = None
                        prev_ptp = None
                        for jt in range(njt):
                            if jt + 2 < njt:
                                pts[jt + 2] = emit_pss(jt + 2)
                            pt = pts.pop(jt)
                            qd2 = jt - 4 * c if causal else -1
                            voff = 128 * qd2 if qd2 > 0 else 0
                            nc.tensor.matmul(
                                pso[:, voff:],
                                vsb[:, jt, h * HD:(h + 1) * HD],
                                pt[:, voff:],
                                start=(jt == 0),
                                stop=(jt == njt - 1),
                            )
                            # denominator: sum pt quads on DVE (2x SBUF mode)
                            # and run the ones-matmul once per four tiles —
                            # quarters the PE cost of the d accumulation and,
                            # more importantly, halves the pipeline bubbles the
                            # 1-row ones-matmul punches into the MM stream
                            # (njt is always a multiple of 4)
                            if jt % 2 == 0:
                                prev_pt = pt
                            else:
                                ptp = work.tile([128, TCH], BF16, tag="ptp",
                                                bufs=3)
                                nc.vector.tensor_add(ptp[:], prev_pt[:], pt[:])
                                if jt % 4 == 1:
                                    prev_ptp = ptp
                                else:
                                    ptq = work.tile([128, TCH], BF16, tag="ptq")
                                    nc.vector.tensor_add(ptq[:], prev_ptp[:], ptp[:])
                                    nc.tensor.matmul(
                                        psd[:],
                                        ones[:, 0:1],
                                        ptq[:],
                                        start=(jt == 3),
                                        stop=(jt == njt - 1),
                                    )
                            step += 1
                            if pending and spacing and step % spacing == 0:
                                emit_pc_chain(*pending.pop(0))
                        # 1/d via single-op approx reciprocal (~18 bits, way
                        # beyond the bf16 pipeline); exact reciprocal costs
                        # 3.3us and Ln/Exp thrash the ACT table
                        drc = sml.tile([1, TCH], F32, tag="drc")
                        nc.vector.reciprocal_approx_fast(drc[:], psd[:])
                        bc = sml.tile([128, TCH], F32, tag="bc")
                        nc.gpsimd.partition_broadcast(bc[:], drc[:])
                        nc.vector.tensor_mul(otc[:, h, :], pso[:], bc[:])
                    while pending:
                        emit_pc_chain(*pending.pop(0))
                    pending = [(c, lt, oc, otc)
                               for lt in range(4) for oc in range(NCH)]
                # tail drain: alternate the PSUM->SBUF copies between DVE and
                # ScalarE so slot recycling isn't single-engine-latency-bound
                for chain in pending:
                    emit_pc_chain(*chain, alternate=True)
    nc.compile()
    return nc


def _get_built(causal: bool):
    if causal not in _BUILT:
        _BUILT[causal] = _build(causal)
    return _BUILT[causal]


def _diag_masks():
    # masks[jl, q*TCH + ii] = 1 if key (128*q + jl) <= query ii in the chunk
    q = np.arange(4)[:, None, None]
    jl = np.arange(128)[None, :, None]
    ii = np.arange(TCH)[None, None, :]
    m = (ii >= 128 * q + jl).astype(np.float32)        # [4, 128, TCH]
    return np.ascontiguousarray(m.transpose(1, 0, 2).reshape(128, 4 * TCH))


def kernel(x, mask, wqkv, wo):
    global LAST_RESULTS
    import ml_dtypes
    from concourse.bass_utils import run_bass_kernel_spmd

    bf16 = ml_dtypes.bfloat16
    x = np.ascontiguousarray(np.asarray(x, dtype=np.float32))
    wqkv_b = np.asarray(wqkv, dtype=np.float32).astype(bf16)
    wo_b = np.asarray(wo, dtype=np.float32).astype(bf16)
    mask_np = np.asarray(mask).reshape(T, T).astype(bool)
    causal = bool(np.array_equal(mask_np, np.tril(np.ones((T, T), dtype=bool))))

    nc = _get_built(causal)
    masks_arr = _diag_masks().astype(bf16)
    maskT = None
    if not causal:
        maskT = np.ascontiguousarray(mask_np.T.astype(np.float32)).astype(bf16)

    in_maps = []
    for core in range(NCORES):
        b, g = divmod(core, NH)
        xT = np.ascontiguousarray(x[b].T).astype(bf16)
        wq = wqkv_b[:, 0 * H * HD + g * NH * HD:0 * H * HD + (g + 1) * NH * HD]
        wk = wqkv_b[:, 1 * H * HD + g * NH * HD:1 * H * HD + (g + 1) * NH * HD]
        wv = wqkv_b[:, 2 * H * HD + g * NH * HD:2 * H * HD + (g + 1) * NH * HD]
        wqkv_g = np.ascontiguousarray(np.concatenate([wq, wk, wv], axis=1))
        wo_g = np.ascontiguousarray(wo_b[g * NH * HD:(g + 1) * NH * HD, :])
        m = {"xT": xT, "wqkv": wqkv_g, "wo": wo_g, "masks": masks_arr}
        if maskT is not None:
            m["maskT"] = maskT
        in_maps.append(m)

    trace = os.environ.get("ATTN_TRACE", "") not in ("", "0")
    res = run_bass_kernel_spmd(nc, in_maps, core_ids=list(range(NCORES)),
                               trace=trace)
    LAST_RESULTS = res

    acc = np.zeros((B, T, D), dtype=np.float64)
    for core in range(NCORES):
        b = core // NH
        acc[b] += res.results[core]["out"].astype(np.float64)
    return acc.astype(np.float32)


# revision 15
# speedup vs baseline: 1.1739x; 1.0148x over previous
"""Causal multi-head attention (B=2, T=2048, DIM=2048, H=16, HD=128) on 8
Trainium2 NeuronCores.

Sharding: core = 4*b + g  (b = batch 0..1, g = head-group 0..3, 4 heads each).
Each core computes, for its batch b and heads 4g..4g+3:
  QKV projection -> causal attention -> partial out = attn_out @ wo[rows of g]
The host sums the 4 partial outputs per batch (the "all-reduce after wo").

On-device layout avoids every transpose:
  - host passes x[b].T, so projections contract d with d on partitions
  - Q^T/K^T kept as [hd, t] (head dim on partitions)
  - scores computed as S^T = K^T_tile.T @ Q^T  ([j, i] layout)
  - exp via ScalarE; causal masking = multiply diagonal tiles by 0/1 masks
  - P@V computed as O^T via lhsT = V tile (natural [t, hd] layout)
  - denominator via ones-vector matmul, normalization via GPSIMD
    partition-broadcast of 1/d + VectorE multiply
  - wo projection consumes O^T tiles directly as stationary operands
All matmul operands are bf16: same PE streaming rate as float32r, but the
weight loads legalize into separate LDWEIGHTS instructions that the PE's
reorder window overlaps with the previous matmul (float32r must self-load
serially, costing ~77ns per matmul). QKV partial sums across the four
d-tile streaming groups accumulate in fp32 SBUF so the bf16 rounding is
paid once, on the final value.
"""

import math
import os

import numpy as np

B, T, D, H, HD = 2, 2048, 2048, 16, 128
NH = 4            # heads per core
NCORES = 8
TCH = 512         # query-chunk width (moving-operand free size)
NDT = D // 128    # 16 d-tiles (contraction tiles for projections)
NTT = T // 128    # 16 t-tiles
NCH = T // TCH    # 4 query chunks
DQ = 4            # d-tiles per accumulation group (PSUM chain length)
NQ = NDT // DQ    # 4 groups

_BUILT = {}
LAST_RESULTS = None  # BassKernelResults of the most recent kernel() call


def _build(causal: bool):
    import concourse.mybir as mybir
    import concourse.tile as tile
    from concourse import bacc

    F32 = mybir.dt.float32
    BF16 = mybir.dt.bfloat16
    EXP = mybir.ActivationFunctionType.Exp
    scale = 1.0 / math.sqrt(HD)

    nc = bacc.Bacc(None, name="attn")
    xT = nc.dram_tensor("xT", [D, T], BF16, kind="ExternalInput")
    wqkv = nc.dram_tensor("wqkv", [D, 3 * NH * HD], BF16, kind="ExternalInput")
    wo = nc.dram_tensor("wo", [NH * HD, D], BF16, kind="ExternalInput")
    masks = nc.dram_tensor("masks", [128, 4 * TCH], BF16, kind="ExternalInput")
    if not causal:
        maskT = nc.dram_tensor("maskT", [T, T], BF16, kind="ExternalInput")
    out = nc.dram_tensor("out", [T, D], F32, kind="ExternalOutput")

    with tile.TileContext(nc) as tc:
        with (
            tc.tile_pool(name="persist", bufs=1) as persist,
            tc.tile_pool(name="ps3", bufs=3, space="PSUM") as ps3,
            tc.tile_pool(name="pso4", bufs=4, space="PSUM") as pso4,
            tc.tile_pool(name="ps1", bufs=1, space="PSUM") as ps1,
        ):
            # persistent operands for the attention phase
            qkt = persist.tile([128, 8, T], BF16)          # slots 0-3: Q^T heads, 4-7: K^T heads
            vsb = persist.tile([128, NTT, NH * HD], BF16)  # V, [t-tile][local t, head*hd]
            ones_f = persist.tile([128, 1], F32)
            ones = persist.tile([128, 1], BF16)
            nc.vector.memset(ones_f[:], 1.0)
            nc.vector.tensor_copy(ones[:], ones_f[:])
            # dummy broadcast: preload the GpSimd PartitionBroadcast ucode
            # library now (~11us HBM fetch) so the first real normalize
            # doesn't stall the whole attention pipeline on LIBRARY_RELOAD
            warm = persist.tile([128, 1], F32)
            nc.gpsimd.partition_broadcast(warm[:], ones_f[0:1, :])
            # PE warmup: the tensor engine clock ramps toward 2.4GHz only
            # after ~10us of continuous execution; run dummy matmuls during
            # the initial DMA wait so the real chains start at full speed
            dm = persist.tile([128, TCH], BF16)
            nc.vector.memset(dm[:], 0.0)
            pwr = pso4.tile([128, TCH], F32, tag="ps_o")
            for _ in range(24):
                nc.tensor.matmul(pwr[:], dm[:, 0:128], dm[:],
                                 start=True, stop=True)

            # ---- Phase A: QKV projections, streaming x^T / wqkv d-tiles.
            # First groups are small so compute starts early. Partial sums
            # across groups accumulate in fp32 SBUF (exact); the last
            # group's add writes the bf16 copy used by the matmuls. ----
            groups = [2, 6, 8]
            offs = [sum(groups[:i]) for i in range(len(groups))]
            comb_n = [0]
            with (
                tc.tile_pool(name="acc", bufs=1) as accp,
                tc.tile_pool(name="xw", bufs=8) as xw,
            ):
                qkt32 = accp.tile([128, 8, T], F32)
                vsb32 = accp.tile([128, NTT, NH * HD], F32)
                for qg, (off, dq) in enumerate(zip(offs, groups)):
                    first, last = qg == 0, qg == len(groups) - 1
                    xts, wqks, wvs = [], [], []
                    for k in range(dq):
                        di = off + k
                        xt_t = xw.tile([128, T], BF16, tag="xt")
                        nc.sync.dma_start(xt_t[:], xT[di * 128:(di + 1) * 128, :])
                        wv_t = xw.tile([128, NH * HD], BF16, tag="wv", bufs=8)
                        nc.sync.dma_start(wv_t[:],
                                          wqkv[di * 128:(di + 1) * 128,
                                               2 * NH * HD:3 * NH * HD])
                        xts.append(xt_t)
                        wvs.append(wv_t)
                    # wqk after the group's xt/wv: the V chains only need
                    # xt+wv, so the first compute starts ~2.5us earlier
                    for k in range(dq):
                        di = off + k
                        wqk_t = xw.tile([128, 2 * NH * HD], BF16, tag="wqk")
                        nc.sync.dma_start(wqk_t[:],
                                          wqkv[di * 128:(di + 1) * 128, 0:2 * NH * HD])
                        wqks.append(wqk_t)
                    # V first: attention's PV chains need V earliest
                    for tt in range(NTT):
                        ps = ps3.tile([128, NH * HD], F32, tag="ps_s")
                        for k in range(dq):
                            nc.tensor.matmul(
                                ps[:],
                                xts[k][:, tt * 128:(tt + 1) * 128],
                                wvs[k][:],
                                start=(k == 0),
                                stop=(k == dq - 1),
                            )
                        acc = vsb32[:, tt, :]
                        if first:
                            comb_n[0] += 1
                            if comb_n[0] % 2 == 0:
                                nc.scalar.copy(acc, ps[:])
                            else:
                                nc.vector.tensor_copy(acc, ps[:])
                        elif last:
                            nc.vector.tensor_add(vsb[:, tt, :], acc, ps[:])
                        else:
                            nc.vector.tensor_add(acc, acc, ps[:])
                    # Q^T / K^T, ordered so K0/Q0 chunk 0 completes first
                    for tch in range(NCH):
                        for s in (4, 0, 5, 1, 6, 2, 7, 3):
                            ps = ps3.tile([128, TCH], F32, tag="ps_s")
                            for k in range(dq):
                                nc.tensor.matmul(
                                    ps[:],
                                    wqks[k][:, s * 128:(s + 1) * 128],
                                    xts[k][:, tch * TCH:(tch + 1) * TCH],
                                    start=(k == 0),
                                    stop=(k == dq - 1),
                                )
                            acc = qkt32[:, s, tch * TCH:(tch + 1) * TCH]
                            if first:
                                comb_n[0] += 1
                                if comb_n[0] % 2 == 0:
                                    nc.scalar.copy(acc, ps[:])
                                else:
                                    nc.vector.tensor_copy(acc, ps[:])
                            elif last:
                                nc.vector.tensor_add(
                                    qkt[:, s, tch * TCH:(tch + 1) * TCH], acc, ps[:])
                            else:
                                nc.vector.tensor_add(acc, acc, ps[:])

            # Phase B/C pools open after the stream pool closed, reusing its
            # SBUF region. wo loads land in that freed space.
            with (
                tc.tile_pool(name="post", bufs=1) as post,
                tc.tile_pool(name="work", bufs=5) as work,
                tc.tile_pool(name="sml", bufs=2) as sml,
                tc.tile_pool(name="otp", bufs=2) as otp,
                tc.tile_pool(name="outp", bufs=4) as outp,
            ):
                msb = post.tile([128, 4 * TCH], BF16)  # diagonal causal masks
                nc.sync.dma_start(msb[:], masks[:])
                # separate 2D tiles: a DMA into a 3D middle-index slice
                # ([128, 1, D] dst AP) hard-faults the exec unit
                wosb = []
                for et in range(NH):
                    wt_ = post.tile([128, D], BF16, tag=f"wos{et}")
                    nc.sync.dma_start(wt_[:], wo[et * 128:(et + 1) * 128, :])
                    wosb.append(wt_)

                # ---- Phase B+C: attention per chunk, with the previous
                # chunk's wo-projection chains interleaved into the jt loop so
                # the in-order PE never sits on PSUM slot recycling ----
                pc_n = [0]

                def emit_pc_chain(c0, lt, oc, otc0, alternate=False):
                    if alternate and pc_n[0] % 2 == 0:
                        ps = ps3.tile([128, TCH], F32, tag="ps_s")
                    else:
                        ps = pso4.tile([128, TCH], F32, tag="ps_o")
                    for h2 in range(NH):
                        nc.tensor.matmul(
                            ps[:],
                            otc0[:, h2, lt * 128:(lt + 1) * 128],
                            wosb[h2][:, oc * TCH:(oc + 1) * TCH],
                            start=(h2 == 0),
                            stop=(h2 == NH - 1),
                        )
                    ost = outp.tile([128, TCH], F32, tag="ost")
                    pc_n[0] += 1
                    if alternate and pc_n[0] % 2 == 0:
                        nc.scalar.copy(ost[:], ps[:])
                    else:
                        nc.vector.tensor_copy(ost[:], ps[:])
                    nc.sync.dma_start(
                        out[(4 * c0 + lt) * 128:(4 * c0 + lt + 1) * 128,
                            oc * TCH:(oc + 1) * TCH],
                        ost[:],
                    )

                pending = []
                for c in range(NCH):
                    otc = otp.tile([128, NH, TCH], BF16, tag="ot")
                    njt = 4 * (c + 1) if causal else NTT
                    steps_total = NH * njt
                    spacing = max(1, steps_total // len(pending)) if pending else 0
                    step = 0
                    for h in range(NH):
                        pso = pso4.tile([128, TCH], F32, tag="ps_o")
                        psd = ps1.tile([1, TCH], F32, tag="ps_d")

                        def emit_pss(jt):
                            # diagonal tiles: queries below 128*qd are fully
                            # masked for this key tile — skip them in the
                            # score matmul. The exp still runs full-width
                            # (stale psum is finite; the mask mul zeroes it).
                            qd = jt - 4 * c if causal else -1
                            off = 128 * qd if qd > 0 else 0
                            pss = ps3.tile([128, TCH], F32, tag="ps_s")
                            nc.tensor.matmul(
                                pss[:, off:],
                                qkt[:, 4 + h, jt * 128:(jt + 1) * 128],
                                qkt[:, h, c * TCH + off:(c + 1) * TCH],
                                start=True,
                                stop=True,
                            )
                            pt = work.tile([128, TCH], BF16, tag="pt")
                            nc.scalar.activation(pt[:], pss[:], EXP, scale=scale)
                            if causal:
                                if qd >= 0:
                                    nc.vector.tensor_mul(pt[:], pt[:], msb[:, qd * TCH:(qd + 1) * TCH])
                            else:
                                mt = work.tile([128, TCH], BF16, tag="mt")
                                nc.sync.dma_start(
                                    mt[:],
                                    maskT[jt * 128:(jt + 1) * 128,
                                          c * TCH:(c + 1) * TCH],
                                )
                                nc.vector.tensor_mul(pt[:], pt[:], mt[:])
                            return pt

                        pts = {}
                        for jt in range(min(2, njt)):
                            pts[jt] = emit_pss(jt)
                        prev_pt = None
                        prev_ptp = None
                        for jt in range(njt):
                            if jt + 2 < njt:
                                pts[jt + 2] = emit_pss(jt + 2)
                            pt = pts.pop(jt)
                            qd2 = jt - 4 * c if causal else -1
                            voff = 128 * qd2 if qd2 > 0 else 0
                            nc.tensor.matmul(
                                pso[:, voff:],
                                vsb[:, jt, h * HD:(h + 1) * HD],
                                pt[:, voff:],
                                start=(jt == 0),
                                stop=(jt == njt - 1),
                            )
                            # denominator: sum pt quads on DVE (2x SBUF mode)
                            # and run the ones-matmul once per four tiles —
                            # quarters the PE cost of the d accumulation and,
                            # more importantly, halves the pipeline bubbles the
                            # 1-row ones-matmul punches into the MM stream
                            # (njt is always a multiple of 4)
                            if jt % 2 == 0:
                                prev_pt = pt
                            else:
                                ptp = work.tile([128, TCH], BF16, tag="ptp",
                                                bufs=3)
                                nc.vector.tensor_add(ptp[:], prev_pt[:], pt[:])
                                if jt % 4 == 1:
                                    prev_ptp = ptp
                                else:
                                    ptq = work.tile([128, TCH], BF16, tag="ptq")
                                    nc.vector.tensor_add(ptq[:], prev_ptp[:], ptp[:])
                                    nc.tensor.matmul(
                                        psd[:],
                                        ones[:, 0:1],
                                        ptq[:],
                                        start=(jt == 3),
                                        stop=(jt == njt - 1),
                                    )
                            step += 1
                            if pending and spacing and step % spacing == 0:
                                emit_pc_chain(*pending.pop(0))
                        # 1/d via single-op approx reciprocal (~18 bits, way
                        # beyond the bf16 pipeline); exact reciprocal costs
                        # 3.3us and Ln/Exp thrash the ACT table
                        drc = sml.tile([1, TCH], F32, tag="drc")
                        nc.vector.reciprocal_approx_fast(drc[:], psd[:])
                        bc = sml.tile([128, TCH], F32, tag="bc")
                        nc.gpsimd.partition_broadcast(bc[:], drc[:])
                        nc.vector.tensor_mul(otc[:, h, :], pso[:], bc[:])
                    while pending:
                        emit_pc_chain(*pending.pop(0))
                    pending = [(c, lt, oc, otc)
                               for lt in range(4) for oc in range(NCH)]
                # tail drain: alternate the PSUM->SBUF copies between DVE and
                # ScalarE so slot recycling isn't single-engine-latency-bound
                for chain in pending:
                    emit_pc_chain(*chain, alternate=True)
    nc.compile()
    return nc


def _get_built(causal: bool):
    if causal not in _BUILT:
        _BUILT[causal] = _build(causal)
    return _BUILT[causal]


def _diag_masks():
    # masks[jl, q*TCH + ii] = 1 if key (128*q + jl) <= query ii in the chunk
    q = np.arange(4)[:, None, None]
    jl = np.arange(128)[None, :, None]
    ii = np.arange(TCH)[None, None, :]
    m = (ii >= 128 * q + jl).astype(np.float32)        # [4, 128, TCH]
    return np.ascontiguousarray(m.transpose(1, 0, 2).reshape(128, 4 * TCH))


def kernel(x, mask, wqkv, wo):
    global LAST_RESULTS
    import ml_dtypes
    from concourse.bass_utils import run_bass_kernel_spmd

    bf16 = ml_dtypes.bfloat16
    x = np.ascontiguousarray(np.asarray(x, dtype=np.float32))
    wqkv_b = np.asarray(wqkv, dtype=np.float32).astype(bf16)
    wo_b = np.asarray(wo, dtype=np.float32).astype(bf16)
    mask_np = np.asarray(mask).reshape(T, T).astype(bool)
    causal = bool(np.array_equal(mask_np, np.tril(np.ones((T, T), dtype=bool))))

    nc = _get_built(causal)
    masks_arr = _diag_masks().astype(bf16)
    maskT = None
    if not causal:
        maskT = np.ascontiguousarray(mask_np.T.astype(np.float32)).astype(bf16)

    in_maps = []
    for core in range(NCORES):
        b, g = divmod(core, NH)
        xT = np.ascontiguousarray(x[b].T).astype(bf16)
        wq = wqkv_b[:, 0 * H * HD + g * NH * HD:0 * H * HD + (g + 1) * NH * HD]
        wk = wqkv_b[:, 1 * H * HD + g * NH * HD:1 * H * HD + (g + 1) * NH * HD]
        wv = wqkv_b[:, 2 * H * HD + g * NH * HD:2 * H * HD + (g + 1) * NH * HD]
        wqkv_g = np.ascontiguousarray(np.concatenate([wq, wk, wv], axis=1))
        wo_g = np.ascontiguousarray(wo_b[g * NH * HD:(g + 1) * NH * HD, :])
        m = {"xT": xT, "wqkv": wqkv_g, "wo": wo_g, "masks": masks_arr}
        if maskT is not None:
            m["maskT"] = maskT
        in_maps.append(m)

    trace = os.environ.get("ATTN_TRACE", "") not in ("", "0")
    res = run_bass_kernel_spmd(nc, in_maps, core_ids=list(range(NCORES)),
                               trace=trace)
    LAST_RESULTS = res

    acc = np.zeros((B, T, D), dtype=np.float64)
    for core in range(NCORES):
        b = core // NH
        acc[b] += res.results[core]["out"].astype(np.float64)
    return acc.astype(np.float32)
